# revision 1
# baseline (speedup 1.0000x reference)
"""Trainium2 Bass kernel for nn_AttentionBlock_223338299515.

Reference (B=4, C=128, H=W=64, N=4096 tokens, 4 heads, d_k=32):
  xs = x.reshape(B,C,N).T ; qkv = xs @ Wp.T + bp ; q,k,v = split(qkv)
  attn = softmax_over_queries(q k^T / sqrt(dk)) ; res = attn-weighted v
  out = (res @ Wo.T + bo + xs).T -> (B, C, H, W)

8 NeuronCores, SPMD: core = (batch b = core//2, head pair hp = core%2).
All math in channel-transposed layout (x[b] used directly as (C, N)):
  qkvT = WpT_rep.T @ x[b]                  (f32r matmuls; Q/K replicated 4x
                                            along partitions -> K=128 S-matmuls)
  S^T[j,i] = K^T.T @ Q^T                   (bf16, equals 4*q.k, folded in scale)
  P[j,i] = exp(S^T * scale/4)              (ScalarE, bf16, accum_out -> Z[j])
  U[j,c] = (V^T.T @ WoT_h) / Z[j]          (merged attn-out + out-projection)
  out^T[c,i] = sum_{h,j} U[j,c] P[j,i]     (+ gamma * x[b] residual on hp==0)
Host: out[b] = core(2b) + core(2b+1) + bo.

The emission order zippers out-matmul groups and next-head projections between
S/exp units so the PE and ACT engine FIFOs stay interleaved, and keeps all
matmuls full-K so the PE HAM clock gate stays at 2.4 GHz.
"""
import os
import sys

import numpy as np

for _p in ("/opt/trn_rl_repo", "/root/.axon_site/_ro/trn_rl_repo"):
    if os.path.isdir(_p) and _p not in sys.path:
        sys.path.insert(0, _p)

import numpy as np
import sys
sys.path.insert(0, "/opt/trn_rl_repo")

import concourse.bacc as bacc
import concourse.tile as tile
from concourse import mybir
from concourse import bass_utils

F32 = mybir.dt.float32
F32R = mybir.dt.float32r
BF16 = mybir.dt.bfloat16
EXP = mybir.ActivationFunctionType.Exp
ADD = mybir.AluOpType.add
MULT = mybir.AluOpType.mult

N = 4096
C = 128
DK = 32
SCALE = float(DK) ** -0.5
NSB = 8          # superblocks per head (512 j each)
NST = 4          # strips (128 j) per superblock
NIC = 8          # i-chunks of 512
NICP = 4         # i-chunk pairs of 1024
QK_DT = BF16     # dtype of Q/K storage (S-matmul inputs)


def build_kernel():
    nc = bacc.Bacc("TRN2", target_bir_lowering=False, debug=False)

    xb_d = nc.dram_tensor("xb", (C, N), F32R, kind="ExternalInput")
    wproj_d = nc.dram_tensor("wproj", (C, 576), F32R, kind="ExternalInput")
    wot_d = nc.dram_tensor("wot", (32, 256), F32R, kind="ExternalInput")
    bias_d = nc.dram_tensor("bias", (C, 6), F32, kind="ExternalInput")
    gamma_d = nc.dram_tensor("gamma", (C, 1), F32, kind="ExternalInput")
    out_d = nc.dram_tensor("out", (C, N), F32, kind="ExternalOutput")

    with tile.TileContext(nc) as tc:
        with (
            tc.tile_pool(name="const", bufs=1) as cpool,
            tc.tile_pool(name="qkv", bufs=2) as qkvp,
            tc.tile_pool(name="pbuf", bufs=2) as pbuf,
            tc.tile_pool(name="acc", bufs=1) as accp,
            tc.tile_pool(name="small", bufs=4) as smallp,
            tc.tile_pool(name="ps_s", bufs=1, space="PSUM") as ps_s,
            tc.tile_pool(name="ps_o", bufs=2, space="PSUM") as ps_o,
            tc.tile_pool(name="ps_x", bufs=2, space="PSUM") as ps_x,
        ):
            xb = cpool.tile([C, N], F32R)
            for dc in range(NIC):
                dsl = slice(512 * dc, 512 * (dc + 1))
                nc.sync.dma_start(out=xb[:, dsl], in_=xb_d.ap()[:, dsl])
            wproj = cpool.tile([C, 576], F32R)
            nc.sync.dma_start(out=wproj[:], in_=wproj_d.ap())
            wot = cpool.tile([32, 256], F32R)
            nc.sync.dma_start(out=wot[:], in_=wot_d.ap())
            bias = cpool.tile([C, 6], F32)
            nc.sync.dma_start(out=bias[:], in_=bias_d.ap())
            gamma = cpool.tile([C, 1], F32)
            nc.sync.dma_start(out=gamma[:], in_=gamma_d.ap())

            out_acc = accp.tile([C, N], F32)
            xb_f32 = xb[:].bitcast(F32)

            pending = []
            unit_ctr = [0]

            def emit_out_group(P, U, ic, first, final=False):
                isl = slice(512 * ic, 512 * (ic + 1))
                op = ps_o.tile([C, 512], F32, name="op")
                for g in range(NST):
                    nc.tensor.matmul(
                        op[:],
                        U[:, 128 * g:128 * (g + 1)],
                        P[:, g, isl],
                        start=(g == 0), stop=(g == NST - 1),
                    )
                if first:
                    nc.vector.scalar_tensor_tensor(
                        out=out_acc[:, isl], in0=xb_f32[:, isl],
                        scalar=gamma[:], in1=op[:],
                        op0=MULT, op1=ADD,
                    )
                else:
                    nc.vector.tensor_tensor(
                        out=out_acc[:, isl], in0=out_acc[:, isl],
                        in1=op[:], op=ADD,
                    )
                if final:
                    nc.sync.dma_start(out=out_d.ap()[:, isl],
                                      in_=out_acc[:, isl])

            def drain(k=1):
                for _ in range(k):
                    if pending:
                        pending.pop(0)()

            def alloc_qkv(h):
                # Q/K hold the projection replicated 4x along partitions so
                # S-matmuls contract a full K=128 (4x folded into exp scale;
                # full-array activity keeps the HAM clock gate warm).
                QT = qkvp.tile([C, N], QK_DT, name=f"QT{h}", tag="QT")
                KT = qkvp.tile([C, N], QK_DT, name=f"KT{h}", tag="KT")
                VT = qkvp.tile([32, N], F32R, name=f"VT{h}", tag="VT")
                return QT, KT, VT

            def emit_proj_unit(h, qkv, ic):
                QT, KT, VT = qkv
                csl = slice(512 * ic, 512 * (ic + 1))
                for qi, dst in enumerate((QT, KT, VT)):
                    rows = 128 if qi < 2 else 32
                    wo = 288 * h + (0, 128, 256)[qi]
                    pj = ps_x.tile([rows, 512], F32, name=f"proj{qi}",
                                   tag="scratch")
                    nc.tensor.matmul(
                        pj[:],
                        wproj[:, wo: wo + rows],
                        xb[:, csl],
                        start=True, stop=True,
                    )
                    nc.vector.tensor_scalar(
                        out=dst[0:rows, csl], in0=pj[:],
                        scalar1=bias[0:rows, 3 * h + qi: 3 * h + qi + 1],
                        scalar2=None, op0=ADD,
                    )

            next_qkv = alloc_qkv(0)
            emit_proj_unit(0, next_qkv, 0)
            emit_proj_unit(0, next_qkv, 1)
            proj_done = 2

            for h in range(2):
                prevPU = []
                QT, KT, VT = cur_qkv = next_qkv

                for sb in range(NSB):
                    if sb == 4 and h == 0:
                        next_qkv = alloc_qkv(1)
                        proj_done = 0
                    # S strips + exp -> P [128, strip, 4096] bf16, Z partials
                    P = pbuf.tile([C, NST, N], BF16, name=f"P{sb % 2}", tag="P")
                    U = pbuf.tile([C, NST * 128], BF16, name=f"U{sb % 2}",
                                  tag="U")
                    zparts = [smallp.tile([C, NICP], F32, name=f"zp{g}")
                              for g in range(NST)]
                    for g in range(NST):
                        s = sb * NST + g
                        jsl = slice(128 * s, 128 * (s + 1))
                        for icp in range(NICP):
                            unit_ctr[0] += 1
                            if h == 0 and sb == 0 and proj_done < NIC:
                                # rest of head-0 projection; unit icp consumes
                                # chunks 2*icp..2*icp+1, so stay a unit ahead
                                while proj_done < min(NIC, 2 * icp + 4):
                                    emit_proj_unit(0, cur_qkv, proj_done)
                                    proj_done += 1
                            elif (h == 0 and sb in (5, 6) and proj_done < NIC
                                  and unit_ctr[0] % 4 == 0):
                                emit_proj_unit(1, next_qkv, proj_done)
                                proj_done += 1
                            st = ps_s.tile([C, 1024], F32, name=f"s{icp % 2}",
                                           tag=f"s{icp % 2}")
                            for half in range(2):
                                ic = 2 * icp + half
                                nc.tensor.matmul(
                                    st[:, 512 * half: 512 * (half + 1)],
                                    KT[:, jsl],
                                    QT[:, 512 * ic: 512 * (ic + 1)],
                                    start=True, stop=True,
                                )
                            nc.scalar.activation(
                                out=P[:, g, 1024 * icp: 1024 * (icp + 1)],
                                in_=st[:],
                                func=EXP, scale=SCALE / 4.0,
                                accum_out=zparts[g][:, icp:icp + 1],
                            )
                            if unit_ctr[0] % 2 == 0:
                                drain(1)
                        # Z reduce/recip; U = (V^T.T @ WoT_h) / Z
                        zs = smallp.tile([C, 1], F32, name=f"zs{g}")
                        nc.vector.tensor_reduce(
                            out=zs[:], in_=zparts[g][:],
                            axis=mybir.AxisListType.X, op=ADD,
                        )
                        zr = smallp.tile([C, 1], F32, name=f"zr{g}")
                        nc.vector.reciprocal(out=zr[:], in_=zs[:])
                        up = ps_x.tile([C, 128], F32, name=f"u{g}",
                                       tag="scratch")
                        nc.tensor.matmul(
                            up[:],
                            VT[:, jsl],
                            wot[:, 128 * h:128 * (h + 1)],
                            start=True, stop=True,
                        )
                        nc.vector.tensor_scalar(
                            out=U[:, 128 * g:128 * (g + 1)], in0=up[:],
                            scalar1=zr[:], scalar2=None, op0=MULT,
                        )
                    # enqueue this superblock's out groups
                    first_sb = (h == 0 and sb == 0)
                    last_sb = (h == 1 and sb == NSB - 1)
                    for ic in range(NIC):
                        pending.append(
                            lambda P=P, U=U, ic=ic, f=first_sb, fin=last_sb:
                                emit_out_group(P, U, ic, f, fin))

            # tail: remaining out groups (final DMAs inlined per chunk)
            drain(len(pending))

    nc.compile()
    return nc


def shard_inputs(x, Wp, bp, Wo, bo=None):
    B, C_, H, W = x.shape
    xf = x.reshape(B, C_, H * W).astype(np.float32)
    in_maps = []
    for core in range(8):
        b = core // 2
        hp = core % 2
        heads = (2 * hp, 2 * hp + 1)
        wproj = np.empty((C_, 576), dtype=np.float32)
        biasm = np.zeros((C_, 6), dtype=np.float32)
        wot = np.empty((32, 256), dtype=np.float32)
        for hi, h in enumerate(heads):
            for qi in range(3):  # q, k, v
                wslc = Wp[96 * h + 32 * qi: 96 * h + 32 * (qi + 1), :]  # [32, C]
                rows = 128 if qi < 2 else 32
                rep = np.tile(wslc, (rows // 32, 1))                    # [rows, C]
                wo = 288 * hi + (0, 128, 256)[qi]
                wproj[:, wo: wo + rows] = rep.T
                biasm[0:rows, 3 * hi + qi] = np.tile(
                    bp[96 * h + 32 * qi: 96 * h + 32 * (qi + 1)], rows // 32)
            wo_h = Wo[:, 32 * h: 32 * (h + 1)]                          # [C, 32]
            wot[:, 128 * hi: 128 * (hi + 1)] = wo_h.T
        gamma = np.full((C_, 1), 1.0 if hp == 0 else 0.0, dtype=np.float32)
        in_maps.append({
            "xb": np.ascontiguousarray(xf[b]),
            "wproj": wproj,
            "wot": wot,
            "bias": biasm,
            "gamma": gamma,
        })
    return in_maps


def unshard_output(results, x_shape, bo):
    B, C_, H, W = x_shape
    out = np.empty((B, C_, H * W), dtype=np.float32)
    for b in range(B):
        out[b] = results[2 * b]["out"] + results[2 * b + 1]["out"] + bo[:, None]
    return out.reshape(B, C_, H, W)


_NC_CACHE = []


def run(inputs, trace=False, tmpdir=None):
    """Run on 8 cores; returns (full_output, exec_time_ns_or_None)."""
    x = np.asarray(inputs["x"], dtype=np.float32)
    Wp = np.asarray(inputs["Wp"], dtype=np.float32)
    bp = np.asarray(inputs["bp"], dtype=np.float32)
    Wo = np.asarray(inputs["Wo"], dtype=np.float32)
    bo = np.asarray(inputs["bo"], dtype=np.float32)

    if not _NC_CACHE:
        _NC_CACHE.append(build_kernel())
    nc = _NC_CACHE[0]

    in_maps = shard_inputs(x, Wp, bp, Wo)
    kwargs = {}
    if trace:
        import tempfile
        kwargs = dict(trace=True,
                      tmpdir=tmpdir or tempfile.mkdtemp(prefix="attn_tr_"))
    res = bass_utils.run_bass_kernel_spmd(nc, in_maps,
                                          core_ids=list(range(8)), **kwargs)
    out = unshard_output(res.results, x.shape, bo)
    return out, res.exec_time_ns


def kernel(x, Wp, bp, Wo, bo):
    out, _ = run({"x": x, "Wp": Wp, "bp": bp, "Wo": Wo, "bo": bo})
    return out



# revision 7
# speedup vs baseline: 1.1877x; 1.1877x over previous
"""Trainium2 Bass kernel for nn_AttentionBlock_223338299515.

Reference (B=4, C=128, H=W=64, N=4096 tokens, 4 heads, d_k=32):
  xs = x.reshape(B,C,N).T ; qkv = xs @ Wp.T + bp ; q,k,v = split(qkv)
  attn = softmax_over_queries(q k^T / sqrt(dk)) ; res = attn-weighted v
  out = (res @ Wo.T + bo + xs).T -> (B, C, H, W)

8 NeuronCores, SPMD: core = (batch b = core//2, head pair hp = core%2).
All math in channel-transposed layout (x[b] used directly as (C, N)):
  S^T[j,i] = x_j^T (Wq^T Wk) x_i = sum_c G[c,j] x[c,i],  G = M^T x
    (M = Wq_h^T Wk_h is folded on the host, so Q/K never materialize:
     one G projection per head and x itself is the S-matmul moving side)
  P[j,i] = exp(S^T*scale - c)  (fp8e4; global shift c cancels in P/Z)
  Z[j]   = sum_i P[j,i]
  U[j,c] = (V^T.T @ WoT_h) * 4096/Z[j]   (fp8e4, merged attn-out+out-proj)
  out^T[c,i] = sum_{h,j} U[j,c] P[j,i] / 4096  (+ gamma*x residual)
Host: out[b] = core(2b) + core(2b+1) + bo.

qkv-bias folding: S^T gains (u.x_i) + (w.x_j + bq.bk) with u = Wk^T bq,
w = Wq^T bk.  The per-j part is constant along the softmax axis (i) and
cancels in P/Z, so only u survives — added per-partition during the
G-copy.  v-bias is added per-partition during the V-copy.  So one kernel
handles any bp.

The exp over the 33.5M-element S matrix is the bottleneck: ACT computes
most chunks natively to fp8 (accum_out gives Z for free); a share is
offloaded to DVE as a Schraudolph bit-trick exp (round(s*8/ln2 + B)
saturated to uint8, bitcast fp8e4) plus a DVE copy-with-accumulate for
those chunks' Z partials.  GPSIMD cannot touch PSUM, so it only does the
residual init.  Out-matmuls are fp8 DoubleRow (K=256, 0.5 cyc/col) over
strip pairs, accumulating 4 superblocks per PSUM tile before one DVE
read-modify-write into out_acc.
NOTE: HW fp8e4 encodes inf/NaN at exponent 15 (max normal 240, unlike
e4m3fn's 448) — the shift c keeps every fp8 value below 240.
"""
import os
import sys

import numpy as np

for _p in ("/opt/trn_rl_repo", "/root/.axon_site/_ro/trn_rl_repo"):
    if os.path.isdir(_p) and _p not in sys.path:
        sys.path.insert(0, _p)

import concourse.bacc as bacc
import concourse.tile as tile
from concourse import mybir
from concourse import bass_utils

F32 = mybir.dt.float32
F32R = mybir.dt.float32r
BF16 = mybir.dt.bfloat16
FP8 = mybir.dt.float8e4
U8 = mybir.dt.uint8
EXP = mybir.ActivationFunctionType.Exp
ADD = mybir.AluOpType.add
MULT = mybir.AluOpType.mult
DR = mybir.MatmulPerfMode.DoubleRow

N = 4096
C = 128
DK = 32
SCALE = float(DK) ** -0.5
NCH = 4            # 1024-col i-chunks per strip
NIC = 8            # 512-col out chunks
WSB = 4            # superblocks per window
NW = 8 // WSB      # windows per head
WST = 4 * WSB      # strips per window

CSHIFT = 1.77      # global logit shift; cancels in P/Z, keeps fp8 < 240
LN2 = float(np.log(2.0))
A8 = 8.0 / LN2
SC8 = SCALE * A8                              # Schraudolph scale on raw S
B8 = (7.0 - 0.0579) * 8.0 - CSHIFT * A8       # Schraudolph bias
USCALE = 4096.0

# exp engine schedule: True -> offload to DVE Schraudolph. Bresenham over
# a 64-unit period; 13/64 offloaded -> ACT 204 chunks, DVE 52.
NUM_O = 13
PAT_O = [((u * NUM_O) % 64) < NUM_O for u in range(64)]


def build_kernel():
    nc = bacc.Bacc("TRN2", target_bir_lowering=False, debug=False)

    xb_d = nc.dram_tensor("xb", (C, N), BF16, kind="ExternalInput")
    wproj_d = nc.dram_tensor("wproj", (C, 320), BF16, kind="ExternalInput")
    wot_d = nc.dram_tensor("wot", (C, 256), F32R, kind="ExternalInput")
    ub_d = nc.dram_tensor("ub", (C, 2), F32, kind="ExternalInput")
    vb_d = nc.dram_tensor("vb", (C, 2), F32, kind="ExternalInput")
    gamma_d = nc.dram_tensor("gamma", (C, 1), F32, kind="ExternalInput")
    out_d = nc.dram_tensor("out", (C, N), F32, kind="ExternalOutput")

    with tile.TileContext(nc) as tc:
        with (
            tc.tile_pool(name="const", bufs=1) as cpool,
            tc.tile_pool(name="gt", bufs=2) as gtp,
            tc.tile_pool(name="vt", bufs=2) as vtp,
            tc.tile_pool(name="pw", bufs=2) as ppool,
            tc.tile_pool(name="upair", bufs=16) as upool,
            tc.tile_pool(name="zp", bufs=6) as zpp,
            tc.tile_pool(name="zs", bufs=6) as zsp,
            tc.tile_pool(name="zr", bufs=8) as zrp,
            tc.tile_pool(name="scr", bufs=2) as scrp,
            tc.tile_pool(name="st", bufs=3, space="PSUM") as stp,
            tc.tile_pool(name="po", bufs=2, space="PSUM") as pso,
        ):
            xb = cpool.tile([C, N], BF16)
            for dc in range(4):
                dsl = slice(1024 * dc, 1024 * (dc + 1))
                nc.sync.dma_start(out=xb[:, dsl], in_=xb_d.ap()[:, dsl])
            wproj = cpool.tile([C, 320], BF16)
            nc.sync.dma_start(out=wproj[:], in_=wproj_d.ap())
            wot = cpool.tile([C, 256], F32R)
            nc.sync.dma_start(out=wot[:], in_=wot_d.ap())
            ub = cpool.tile([C, 2], F32)
            nc.sync.dma_start(out=ub[:], in_=ub_d.ap())
            vb = cpool.tile([C, 2], F32)
            nc.sync.dma_start(out=vb[:], in_=vb_d.ap())
            gamma = cpool.tile([C, 1], F32)
            nc.sync.dma_start(out=gamma[:], in_=gamma_d.ap())
            bias_t = cpool.tile([C, 1], F32)
            nc.vector.memset(bias_t[:], -CSHIFT)
            out_acc = cpool.tile([C, N], F32)

            # residual init: out_acc = gamma * x (gamma is 1 or 0)
            for dc in range(8):
                dsl = slice(512 * dc, 512 * (dc + 1))
                nc.gpsimd.tensor_scalar(
                    out=out_acc[:, dsl], in0=xb[:, dsl],
                    scalar1=gamma[:], scalar2=None, op0=MULT)

            pending = []

            def drain(k=1):
                for _ in range(k):
                    if pending:
                        pending.pop(0)()

            def alloc_gv(h):
                GT = gtp.tile([C, N], BF16, name=f"GT{h}", tag="GT")
                # V packed on partition quadrants: rows 32q..32q+31 hold
                # v-cols 1024q..1024q+1023 (wot is replicated to match).
                VT = vtp.tile([C, N // 4], F32R, name=f"VT{h}", tag="VT")
                return GT, VT

            def emit_gproj(h, gv, dc):
                GT, _ = gv
                pj = stp.tile([C, 1024], F32, name="pj", tag="st")
                for half in range(2):
                    xsl = slice(1024 * dc + 512 * half,
                                1024 * dc + 512 * (half + 1))
                    nc.tensor.matmul(
                        pj[:, 512 * half: 512 * (half + 1)],
                        wproj[:, 160 * h: 160 * h + 128], xb[:, xsl],
                        start=True, stop=True)
                csl = slice(1024 * dc, 1024 * (dc + 1))
                nc.vector.tensor_scalar(
                    out=GT[:, csl], in0=pj[:],
                    scalar1=ub[:, h: h + 1], scalar2=None, op0=ADD)

            def emit_vproj(h, gv):
                _, VT = gv
                pj = stp.tile([C, 1024], F32, name="pj", tag="st")
                for q in range(4):
                    for half in range(2):
                        xsl = slice(1024 * q + 512 * half,
                                    1024 * q + 512 * (half + 1))
                        nc.tensor.matmul(
                            pj[32 * q: 32 * (q + 1),
                               512 * half: 512 * (half + 1)],
                            wproj[:, 160 * h + 128: 160 * h + 160],
                            xb[:, xsl],
                            start=True, stop=True,
                            tile_position=(0, 32 * q))
                nc.vector.tensor_scalar(
                    out=VT[:], in0=pj[:],
                    scalar1=vb[:, h: h + 1], scalar2=None, op0=ADD)

            def emit_out_group(P, Upairs, ic, final):
                isl = slice(512 * ic, 512 * (ic + 1))
                op = pso.tile([C, 512], F32, name="op", tag="po")
                npair = WST // 2
                for t in range(npair):
                    nc.tensor.matmul(
                        op[:], Upairs[t][:], P[:, 2 * t: 2 * t + 2, isl],
                        start=(t == 0), stop=(t == npair - 1), perf_mode=DR)
                nc.vector.scalar_tensor_tensor(
                    out=out_acc[:, isl], in0=op[:], scalar=1.0 / USCALE,
                    in1=out_acc[:, isl], op0=MULT, op1=ADD)
                if final:
                    nc.sync.dma_start(out=out_d.ap()[:, isl],
                                      in_=out_acc[:, isl])

            cur_gv = alloc_gv(0)
            next_gv = None
            emit_gproj(0, cur_gv, 0)
            emit_vproj(0, cur_gv)

            for h in range(2):
                GT, VT = cur_gv
                for w in range(NW):
                    P = ppool.tile([C, WST, N], FP8, name=f"P{w % 2}",
                                   tag="P")
                    Upairs = [upool.tile([C, 2, C], FP8, name=f"U{t}",
                                         tag="U") for t in range(WST // 2)]
                    for row in range(WST):
                        sidx = w * WST + row
                        jsl = slice(128 * sidx, 128 * (sidx + 1))
                        zparts = zpp.tile([C, NCH], F32, name="zparts")
                        for cch in range(NCH):
                            unit = sidx * NCH + cch
                            if unit in (26, 58, 90):
                                emit_gproj(h, cur_gv, (unit + 6) // 32)
                            elif h == 0 and unit == 104:
                                next_gv = alloc_gv(1)
                                emit_gproj(1, next_gv, 0)
                            elif h == 0 and unit == 116:
                                emit_vproj(1, next_gv)
                            csl = slice(1024 * cch, 1024 * (cch + 1))
                            st = stp.tile([C, 1024], F32, name="st", tag="st")
                            for half in range(2):
                                ic = 2 * cch + half
                                nc.tensor.matmul(
                                    st[:, 512 * half: 512 * (half + 1)],
                                    GT[:, jsl],
                                    xb[:, 512 * ic: 512 * (ic + 1)],
                                    start=True, stop=True)
                            zslot = zparts[:, cch: cch + 1]
                            if PAT_O[unit % 64]:
                                nc.vector.tensor_scalar(
                                    out=P[:, row, csl].bitcast(U8),
                                    in0=st[:], scalar1=SC8, scalar2=B8,
                                    op0=MULT, op1=ADD)
                                scr = scrp.tile([C, 1024], FP8, name="scr")
                                nc.vector.tensor_scalar(
                                    out=scr[:], in0=P[:, row, csl],
                                    scalar1=1.0, scalar2=0.0,
                                    op0=MULT, op1=ADD, accum_out=zslot)
                            else:
                                nc.scalar.activation(
                                    out=P[:, row, csl], in_=st[:],
                                    func=EXP, scale=SCALE,
                                    bias=bias_t[:], accum_out=zslot)
                            if unit % 8 == 7:
                                drain(1)
                        # strip tail: Z, U
                        zs = zsp.tile([C, 1], F32, name="zs")
                        nc.vector.tensor_reduce(
                            out=zs[:], in_=zparts[:],
                            axis=mybir.AxisListType.X, op=ADD)
                        zr = zrp.tile([C, 1], F32, name="zr")
                        nc.vector.reciprocal(out=zr[:], in_=zs[:])
                        vq = sidx // 8
                        vcl = slice(128 * (sidx % 8), 128 * (sidx % 8 + 1))
                        up = pso.tile([C, C], F32, name="up", tag="po")
                        nc.tensor.matmul(
                            up[:], VT[32 * vq: 32 * (vq + 1), vcl],
                            wot[32 * vq: 32 * (vq + 1),
                                128 * h: 128 * (h + 1)],
                            start=True, stop=True,
                            tile_position=(32 * vq, 0))
                        nc.vector.tensor_scalar(
                            out=Upairs[row // 2][:, row % 2, :], in0=up[:],
                            scalar1=zr[:], scalar2=USCALE,
                            op0=MULT, op1=MULT)
                    final = (h == 1 and w == NW - 1)
                    for ic in range(NIC):
                        pending.append(
                            lambda P=P, U=Upairs, ic=ic, fin=final:
                                emit_out_group(P, U, ic, fin))
                cur_gv = next_gv
            drain(len(pending))

    nc.compile()
    return nc


def shard_inputs(x, Wp, bp, Wo):
    import ml_dtypes
    B, C_, H, W = x.shape
    xf = x.reshape(B, C_, H * W).astype(np.float32)
    in_maps = []
    for core in range(8):
        b = core // 2
        hp = core % 2
        heads = (2 * hp, 2 * hp + 1)
        wproj = np.empty((C_, 320), dtype=np.float32)
        ub = np.zeros((C_, 2), dtype=np.float32)
        vb = np.zeros((C_, 2), dtype=np.float32)
        wot = np.empty((32, 256), dtype=np.float32)
        for hi, h in enumerate(heads):
            Wq = Wp[96 * h: 96 * h + 32, :]          # (32, C)
            Wk = Wp[96 * h + 32: 96 * h + 64, :]
            Wv = Wp[96 * h + 64: 96 * h + 96, :]
            bq = bp[96 * h: 96 * h + 32]
            bk = bp[96 * h + 32: 96 * h + 64]
            bv = bp[96 * h + 64: 96 * h + 96]
            wproj[:, 160 * hi: 160 * hi + 128] = Wk.T @ Wq   # M^T
            wproj[:, 160 * hi + 128: 160 * hi + 160] = Wv.T
            ub[:, hi] = Wq.T @ bk       # u: survives along the i axis
            vb[:, hi] = np.tile(bv, 4)
            wo_h = Wo[:, 32 * h: 32 * (h + 1)]
            wot[:, 128 * hi: 128 * (hi + 1)] = wo_h.T
        gamma = np.full((C_, 1), 1.0 if hp == 0 else 0.0, dtype=np.float32)
        in_maps.append({
            "xb": np.ascontiguousarray(xf[b]).astype(ml_dtypes.bfloat16),
            "wproj": wproj.astype(ml_dtypes.bfloat16),
            "wot": np.ascontiguousarray(np.tile(wot, (4, 1))),
            "ub": ub,
            "vb": vb,
            "gamma": gamma,
        })
    return in_maps


def unshard_output(results, x_shape, bo):
    B, C_, H, W = x_shape
    out = np.empty((B, C_, H * W), dtype=np.float32)
    for b in range(B):
        out[b] = results[2 * b]["out"] + results[2 * b + 1]["out"] \
            + bo[:, None]
    return out.reshape(B, C_, H, W)


_NC_CACHE = []


def run(inputs, trace=False, tmpdir=None):
    """Run on 8 cores; returns (full_output, exec_time_ns_or_None)."""
    x = np.asarray(inputs["x"], dtype=np.float32)
    Wp = np.asarray(inputs["Wp"], dtype=np.float32)
    bp = np.asarray(inputs["bp"], dtype=np.float32)
    Wo = np.asarray(inputs["Wo"], dtype=np.float32)
    bo = np.asarray(inputs["bo"], dtype=np.float32)

    if not _NC_CACHE:
        _NC_CACHE.append(build_kernel())
    nc = _NC_CACHE[0]

    in_maps = shard_inputs(x, Wp, bp, Wo)
    kwargs = {}
    if trace:
        import tempfile
        kwargs = dict(trace=True,
                      tmpdir=tmpdir or tempfile.mkdtemp(prefix="attn_tr_"))
    res = bass_utils.run_bass_kernel_spmd(nc, in_maps,
                                          core_ids=list(range(8)), **kwargs)
    out = unshard_output(res.results, x.shape, bo)
    return out, res.exec_time_ns


def kernel(x, Wp, bp, Wo, bo):
    out, _ = run({"x": x, "Wp": Wp, "bp": bp, "Wo": Wo, "bo": bo})
    return out
